# revision 21
# baseline (speedup 1.0000x reference)
"""MaxPool1d(K=4, stride=2, VALID) over ragged NaN-padded sequences.

Full input:  x  [16, 8, 64, 8192] f32, channel c valid prefix LENS[c], NaN tail.
Full output:    [16, 8, 64, 4095] f32, pooled valid prefix, NaN tail.

Sharding: data-parallel over batch — 16 batches / 8 cores = 2 per core.

Layout trick: adjacent channel PAIRS (0,1), (2,3), (4,5), (6,7) are
contiguous in DRAM for a fixed batch, so a [128, W] SBUF tile whose
partition dim is (2 channels x 64 features) maps to a fully CONTIGUOUS
DRAM block — strided row reads/writes (~3x slower on the SDMA engines)
disappear:
  - big pairs (0,1)/(4,5): load the full [128, 8192] block (reading the
    NaN tail of the odd channel costs less than a strided prefix read),
  - small pairs (2,3)/(6,7): load [128, 2048] (prefix of both channels,
    strided, but only 1 MB),
  - every store is one contiguous [128, 4095] block: valid prefix computed
    into persistent per-pair tiles whose NaN tails are pre-memset once.
Pooling per pair-tile (128-wide stage 1, per-64-row-half stage 2):
  stage 1: m[i] = max(x[2i], x[2i+1])   stride-2 tensor_tensor max
  stage 2: out[p] = max(m[p], m[p+1])   unit-stride
All DMA via gpsimd SWDGE (HWDGE dynamic rings are serial ~27 GB/s); the
emission order is software-pipelined [.., compute k, load k+3, store k, ..]
so a store's compute-wait never delays the next load's descriptor
emission. Memsets run on GpSimd: fp32 1x-mode DVE MAXes never contend
with GpSimd, and DVE 2-port memsets could stall SWDGE descriptor writes.
"""

import numpy as np

# ---- problem constants (hardcoded; kernel.py must be self-contained) ----
B, C, F, S = 16, 8, 64, 8192
K, STRIDE = 4, 2
P_OUT = (S - K) // STRIDE + 1  # 4095
LENS = [8192, 4096, 2048, 1024, 8192, 4096, 2048, 1024]
N_CORES = 8
B_LOC = B // N_CORES  # 2 batches per core

_CACHE = {}


def _pv(L):
    return (L - K) // STRIDE + 1


def _build_nc():
    import concourse.bacc as bacc
    import concourse.mybir as mybir
    from concourse.tile import TileContext

    f32 = mybir.dt.float32
    nc = bacc.Bacc("TRN2", debug=False, num_devices=N_CORES)
    x = nc.declare_dram_parameter("x", [B_LOC, C, F, S], f32, isOutput=False)
    out = nc.declare_dram_parameter("out", [B_LOC, C, F, P_OUT], f32, isOutput=True)
    x_ap = x.ap()
    out_ap = out.ap()

    # pair base channel -> (columns to load, full-contiguous?)
    PAIRS = {0: (8192, True), 4: (8192, True), 2: (2048, False), 6: (2048, False)}
    WORK = [(0, 0), (4, 0), (0, 1), (4, 1), (2, 0), (6, 0), (2, 1), (6, 1)]
    AHEAD = 3  # loads emitted ahead of the compute/store steady state

    with TileContext(nc) as tc:
        with tc.tile_pool(name="big", bufs=3) as big_pool, \
             tc.tile_pool(name="small", bufs=3) as small_pool, \
             tc.tile_pool(name="midb", bufs=1) as midb_pool, \
             tc.tile_pool(name="mids", bufs=1) as mids_pool, \
             tc.tile_pool(name="res", bufs=1) as res_pool:

            def emit_load(cp, b):
                W, full = PAIRS[cp]
                pool = big_pool if full else small_pool
                xin = pool.tile([128, W], f32, tag="big" if full else "small")
                if full:
                    # even channel: full row, contiguous 2 MB block; odd
                    # channel: valid prefix only (strided, skips the 1 MB
                    # NaN tail the old full-pair load used to read)
                    L1 = LENS[cp + 1]
                    nc.gpsimd.dma_start(
                        out=xin[0:64, :], in_=x_ap[b, cp, :, :]
                    )
                    nc.gpsimd.dma_start(
                        out=xin[64:128, 0:L1], in_=x_ap[b, cp + 1, :, 0:L1]
                    )
                else:
                    nc.gpsimd.dma_start(out=xin[:], in_=x_ap[b, cp:cp + 2, :, 0:W])
                return xin

            xins = {}
            for cp, b in WORK[:AHEAD]:
                xins[(cp, b)] = emit_load(cp, b)

            # persistent per-pair output tiles with NaN tails pre-set (GpSimd)
            otiles = {}
            for cp in PAIRS:
                o = res_pool.tile([128, P_OUT], f32, tag=f"o{cp}")
                for half, c in ((0, cp), (1, cp + 1)):
                    tail0 = _pv(LENS[c])
                    if tail0 < P_OUT:
                        nc.gpsimd.memset(
                            o[64 * half:64 * half + 64, tail0:P_OUT], float("nan")
                        )
                otiles[cp] = o

            for wi, (cp, b) in enumerate(WORK):
                W, full = PAIRS[cp]
                xin = xins.pop((cp, b))

                mpool = midb_pool if full else mids_pool
                m = mpool.tile([128, W // 2], f32, tag="midb" if full else "mids")
                if full:
                    # per-half stage 1: the odd channel's NaN tail was never
                    # loaded, so only pool its valid prefix
                    L1 = LENS[cp + 1]
                    x2a = xin[0:64, :].rearrange("p (n two) -> p n two", two=2)
                    nc.vector.tensor_max(m[0:64, :], x2a[:, :, 0], x2a[:, :, 1])
                    x2b = xin[64:128, 0:L1].rearrange(
                        "p (n two) -> p n two", two=2
                    )
                    nc.vector.tensor_max(
                        m[64:128, 0:L1 // 2], x2b[:, :, 0], x2b[:, :, 1]
                    )
                else:
                    x2 = xin[:].rearrange("p (n two) -> p n two", two=2)
                    nc.vector.tensor_max(m[:], x2[:, :, 0], x2[:, :, 1])

                o = otiles[cp]
                for half, c in ((0, cp), (1, cp + 1)):
                    Pv = _pv(LENS[c])
                    r0 = 64 * half
                    nc.vector.tensor_max(
                        o[r0:r0 + 64, 0:Pv],
                        m[r0:r0 + 64, 0:Pv],
                        m[r0:r0 + 64, 1:Pv + 1],
                    )

                # next load's emission goes ahead of this store in the
                # gpsimd stream
                if wi + AHEAD < len(WORK):
                    nxt = WORK[wi + AHEAD]
                    xins[nxt] = emit_load(*nxt)

                # one contiguous [128, P_OUT] store covering both channels
                nc.gpsimd.dma_start(out=out_ap[b, cp:cp + 2, :, :], in_=o[:])
    nc.compile()
    return nc


def _get_nc():
    if "nc" not in _CACHE:
        _CACHE["nc"] = _build_nc()
    return _CACHE["nc"]


def kernel(x: np.ndarray) -> np.ndarray:
    from concourse.bass_utils import run_bass_kernel_spmd

    x = np.asarray(x, dtype=np.float32)
    assert x.shape == (B, C, F, S), x.shape

    nc = _get_nc()
    in_maps = [
        {"x": np.ascontiguousarray(x[i * B_LOC:(i + 1) * B_LOC])}
        for i in range(N_CORES)
    ]
    res = run_bass_kernel_spmd(nc, in_maps, list(range(N_CORES)))
    return np.concatenate([r["out"] for r in res.results], axis=0)


# revision 23
# speedup vs baseline: 1.0591x; 1.0591x over previous
"""MaxPool1d(K=4, stride=2, VALID) over ragged NaN-padded sequences.

Full input:  x  [16, 8, 64, 8192] f32, channel c valid prefix LENS[c], NaN tail.
Full output:    [16, 8, 64, 4095] f32, pooled valid prefix, NaN tail.

Sharding: data-parallel over batch — 16 batches / 8 cores = 2 per core.

Layout trick: adjacent channel PAIRS (0,1), (2,3), (4,5), (6,7) are
contiguous in DRAM for a fixed batch, so a [128, W] SBUF tile whose
partition dim is (2 channels x 64 features) maps to a fully CONTIGUOUS
DRAM block — strided row reads/writes (~3x slower on the SDMA engines)
disappear:
  - big pairs (0,1)/(4,5): load the full [128, 8192] block (reading the
    NaN tail of the odd channel costs less than a strided prefix read),
  - small pairs (2,3)/(6,7): load [128, 2048] (prefix of both channels,
    strided, but only 1 MB),
  - every store is one contiguous [128, 4095] block: valid prefix computed
    into persistent per-pair tiles whose NaN tails are pre-memset once.
Pooling per pair-tile (128-wide stage 1, per-64-row-half stage 2):
  stage 1: m[i] = max(x[2i], x[2i+1])   stride-2 tensor_tensor max
  stage 2: out[p] = max(m[p], m[p+1])   unit-stride
All DMA via gpsimd SWDGE (HWDGE dynamic rings are serial ~27 GB/s); the
emission order is software-pipelined [.., compute k, load k+3, store k, ..]
so a store's compute-wait never delays the next load's descriptor
emission. Memsets run on GpSimd: fp32 1x-mode DVE MAXes never contend
with GpSimd, and DVE 2-port memsets could stall SWDGE descriptor writes.
"""

import numpy as np

# ---- problem constants (hardcoded; kernel.py must be self-contained) ----
B, C, F, S = 16, 8, 64, 8192
K, STRIDE = 4, 2
P_OUT = (S - K) // STRIDE + 1  # 4095
LENS = [8192, 4096, 2048, 1024, 8192, 4096, 2048, 1024]
N_CORES = 8
B_LOC = B // N_CORES  # 2 batches per core

_CACHE = {}


def _pv(L):
    return (L - K) // STRIDE + 1


def _build_nc():
    import concourse.bacc as bacc
    import concourse.mybir as mybir
    from concourse.tile import TileContext

    f32 = mybir.dt.float32
    nc = bacc.Bacc("TRN2", debug=False, num_devices=N_CORES)
    x = nc.declare_dram_parameter("x", [B_LOC, C, F, S], f32, isOutput=False)
    out = nc.declare_dram_parameter("out", [B_LOC, C, F, P_OUT], f32, isOutput=True)
    x_ap = x.ap()
    out_ap = out.ap()

    # pair base channel -> (columns to load, full-contiguous?)
    PAIRS = {0: (8192, True), 4: (8192, True), 2: (2048, False), 6: (2048, False)}
    WORK = [(0, 0), (4, 0), (0, 1), (4, 1), (2, 0), (6, 0), (2, 1), (6, 1)]
    AHEAD = 3  # loads emitted ahead of the compute/store steady state

    with TileContext(nc) as tc:
        with tc.tile_pool(name="big", bufs=3) as big_pool, \
             tc.tile_pool(name="small", bufs=3) as small_pool, \
             tc.tile_pool(name="midb", bufs=1) as midb_pool, \
             tc.tile_pool(name="mids", bufs=1) as mids_pool, \
             tc.tile_pool(name="res", bufs=1) as res_pool:

            def emit_load(cp, b):
                W, full = PAIRS[cp]
                pool = big_pool if full else small_pool
                xin = pool.tile([128, W], f32, tag="big" if full else "small")
                nc.gpsimd.dma_start(out=xin[:], in_=x_ap[b, cp:cp + 2, :, 0:W])
                return xin

            xins = {}
            for cp, b in WORK[:AHEAD]:
                xins[(cp, b)] = emit_load(cp, b)

            # persistent per-pair output tiles with NaN tails pre-set (GpSimd)
            otiles = {}
            for cp in PAIRS:
                o = res_pool.tile([128, P_OUT], f32, tag=f"o{cp}")
                for half, c in ((0, cp), (1, cp + 1)):
                    tail0 = _pv(LENS[c])
                    if tail0 < P_OUT:
                        nc.gpsimd.memset(
                            o[64 * half:64 * half + 64, tail0:P_OUT], float("nan")
                        )
                otiles[cp] = o

            for wi, (cp, b) in enumerate(WORK):
                W, full = PAIRS[cp]
                xin = xins.pop((cp, b))

                mpool = midb_pool if full else mids_pool
                m = mpool.tile([128, W // 2], f32, tag="midb" if full else "mids")
                x2 = xin[:].rearrange("p (n two) -> p n two", two=2)
                nc.vector.tensor_max(m[:], x2[:, :, 0], x2[:, :, 1])

                o = otiles[cp]
                for half, c in ((0, cp), (1, cp + 1)):
                    Pv = _pv(LENS[c])
                    r0 = 64 * half
                    nc.vector.tensor_max(
                        o[r0:r0 + 64, 0:Pv],
                        m[r0:r0 + 64, 0:Pv],
                        m[r0:r0 + 64, 1:Pv + 1],
                    )

                # next load's emission goes ahead of this store in the
                # gpsimd stream
                if wi + AHEAD < len(WORK):
                    nxt = WORK[wi + AHEAD]
                    xins[nxt] = emit_load(*nxt)

                # one contiguous [128, P_OUT] store covering both channels
                nc.gpsimd.dma_start(out=out_ap[b, cp:cp + 2, :, :], in_=o[:])
    nc.compile()
    return nc


def _get_nc():
    if "nc" not in _CACHE:
        _CACHE["nc"] = _build_nc()
    return _CACHE["nc"]


def kernel(x: np.ndarray) -> np.ndarray:
    from concourse.bass_utils import run_bass_kernel_spmd

    x = np.asarray(x, dtype=np.float32)
    assert x.shape == (B, C, F, S), x.shape

    nc = _get_nc()
    in_maps = [
        {"x": np.ascontiguousarray(x[i * B_LOC:(i + 1) * B_LOC])}
        for i in range(N_CORES)
    ]
    res = run_bass_kernel_spmd(nc, in_maps, list(range(N_CORES)))
    return np.concatenate([r["out"] for r in res.results], axis=0)


# revision 24
# speedup vs baseline: 1.3480x; 1.2728x over previous
"""MaxPool1d(K=4, stride=2, VALID) over ragged NaN-padded sequences.

Full input:  x  [16, 8, 64, 8192] f32, channel c valid prefix LENS[c], NaN tail.
Full output:    [16, 8, 64, 4095] f32, pooled valid prefix, NaN tail.

Sharding: data-parallel over batch — 16 batches / 8 cores = 2 per core.

Layout trick: adjacent channel PAIRS (0,1), (2,3), (4,5), (6,7) are
contiguous in DRAM for a fixed batch, so a [128, W] SBUF tile whose
partition dim is (2 channels x 64 features) maps to a fully CONTIGUOUS
DRAM block — strided row reads/writes (~3x slower on the SDMA engines)
disappear:
  - big pairs (0,1)/(4,5): load the full [128, 8192] block (reading the
    NaN tail of the odd channel costs less than a strided prefix read),
  - small pairs (2,3)/(6,7): load [128, 2048] (prefix of both channels,
    strided, but only 1 MB),
  - every store is one contiguous [128, 4095] block: valid prefix computed
    into persistent per-pair tiles whose NaN tails are pre-memset once.
Pooling per pair-tile (128-wide stage 1, per-64-row-half stage 2):
  stage 1: m[i] = max(x[2i], x[2i+1])   stride-2 tensor_tensor max
  stage 2: out[p] = max(m[p], m[p+1])   unit-stride
All DMA via gpsimd SWDGE (HWDGE dynamic rings are serial ~27 GB/s); the
emission order is software-pipelined [.., compute k, load k+3, store k, ..]
so a store's compute-wait never delays the next load's descriptor
emission. Memsets run on GpSimd: fp32 1x-mode DVE MAXes never contend
with GpSimd, and DVE 2-port memsets could stall SWDGE descriptor writes.
"""

import numpy as np

# ---- problem constants (hardcoded; kernel.py must be self-contained) ----
B, C, F, S = 16, 8, 64, 8192
K, STRIDE = 4, 2
P_OUT = (S - K) // STRIDE + 1  # 4095
LENS = [8192, 4096, 2048, 1024, 8192, 4096, 2048, 1024]
N_CORES = 8
B_LOC = B // N_CORES  # 2 batches per core

_CACHE = {}


def _pv(L):
    return (L - K) // STRIDE + 1


def _build_nc():
    import concourse.bacc as bacc
    import concourse.mybir as mybir
    from concourse.tile import TileContext

    f32 = mybir.dt.float32
    nc = bacc.Bacc("TRN2", debug=False, num_devices=N_CORES)
    x = nc.declare_dram_parameter("x", [B_LOC, C, F, S], f32, isOutput=False)
    out = nc.declare_dram_parameter("out", [B_LOC, C, F, P_OUT], f32, isOutput=True)
    x_ap = x.ap()
    out_ap = out.ap()

    # pair base channel -> (columns to load, full-contiguous?)
    PAIRS = {0: (8192, True), 4: (8192, True), 2: (2048, False), 6: (2048, False)}
    WORK = [(0, 0), (4, 0), (2, 0), (0, 1), (4, 1), (6, 0), (2, 1), (6, 1)]
    AHEAD = 3  # loads emitted ahead of the compute/store steady state

    with TileContext(nc) as tc:
        with tc.tile_pool(name="big", bufs=3) as big_pool, \
             tc.tile_pool(name="small", bufs=3) as small_pool, \
             tc.tile_pool(name="midb", bufs=1) as midb_pool, \
             tc.tile_pool(name="mids", bufs=1) as mids_pool, \
             tc.tile_pool(name="res", bufs=1) as res_pool:

            def emit_load(cp, b):
                W, full = PAIRS[cp]
                pool = big_pool if full else small_pool
                xin = pool.tile([128, W], f32, tag="big" if full else "small")
                nc.gpsimd.dma_start(out=xin[:], in_=x_ap[b, cp:cp + 2, :, 0:W])
                return xin

            xins = {}
            for cp, b in WORK[:AHEAD]:
                xins[(cp, b)] = emit_load(cp, b)

            # persistent per-pair output tiles with NaN tails pre-set (GpSimd)
            otiles = {}
            for cp in PAIRS:
                o = res_pool.tile([128, P_OUT], f32, tag=f"o{cp}")
                for half, c in ((0, cp), (1, cp + 1)):
                    tail0 = _pv(LENS[c])
                    if tail0 < P_OUT:
                        nc.gpsimd.memset(
                            o[64 * half:64 * half + 64, tail0:P_OUT], float("nan")
                        )
                otiles[cp] = o

            for wi, (cp, b) in enumerate(WORK):
                W, full = PAIRS[cp]
                xin = xins.pop((cp, b))

                mpool = midb_pool if full else mids_pool
                m = mpool.tile([128, W // 2], f32, tag="midb" if full else "mids")
                x2 = xin[:].rearrange("p (n two) -> p n two", two=2)
                nc.vector.tensor_max(m[:], x2[:, :, 0], x2[:, :, 1])

                o = otiles[cp]
                for half, c in ((0, cp), (1, cp + 1)):
                    Pv = _pv(LENS[c])
                    r0 = 64 * half
                    nc.vector.tensor_max(
                        o[r0:r0 + 64, 0:Pv],
                        m[r0:r0 + 64, 0:Pv],
                        m[r0:r0 + 64, 1:Pv + 1],
                    )

                # next load's emission goes ahead of this store in the
                # gpsimd stream
                if wi + AHEAD < len(WORK):
                    nxt = WORK[wi + AHEAD]
                    xins[nxt] = emit_load(*nxt)

                # one contiguous [128, P_OUT] store covering both channels
                nc.gpsimd.dma_start(out=out_ap[b, cp:cp + 2, :, :], in_=o[:])
    nc.compile()
    return nc


def _get_nc():
    if "nc" not in _CACHE:
        _CACHE["nc"] = _build_nc()
    return _CACHE["nc"]


def kernel(x: np.ndarray) -> np.ndarray:
    from concourse.bass_utils import run_bass_kernel_spmd

    x = np.asarray(x, dtype=np.float32)
    assert x.shape == (B, C, F, S), x.shape

    nc = _get_nc()
    in_maps = [
        {"x": np.ascontiguousarray(x[i * B_LOC:(i + 1) * B_LOC])}
        for i in range(N_CORES)
    ]
    res = run_bass_kernel_spmd(nc, in_maps, list(range(N_CORES)))
    return np.concatenate([r["out"] for r in res.results], axis=0)
